# revision 43
# baseline (speedup 1.0000x reference)
"""DGL-style multi-head graph attention on 8 Trainium2 NeuronCores.

Strategy (edge/node hybrid parallelism, no collectives needed):
  * Host LPT-balances dst nodes into the 80 (core, tile) bins by degree
    (max 4001 edges/bin -> B=32, ~2.4% padding); each core owns all edges
    landing in its 10 bins, so per-core outputs are disjoint rows.
  * On each core: project q/v tables for all nodes and k for the local
    range (PE matmuls), then per node-tile gather q[src] (transposed),
    v[src], k[dst] (transposed) with dma_gather, scores via folded-half
    BD matmul, softmax-over-heads, message multiply, segment-sum as a
    one-hot matmul in PSUM; Wo projection; per-tile bf16 out writes.
  * Host scatters rows back through the balance permutation and adds bo.

Layout tricks (all host-side, numerically exact):
  * v-table columns (d,h)-permuted + Wo rows to match -> the attn
    broadcast in the message multiply lands mid-dim, last dim packed
    -> 2x DVE mode (the old 1x broadcast was the biggest DVE item).
  * q/k-table columns half-head permuted (head h owns cols h*16..+16 and
    128+h*16..+16) -> qk halves fold with one DVE add and ONE
    128-contraction matmul per 512-chunk computes scoresT (PE halved).
  * xT uploaded pre-arranged [p, a, t, w] so phase-1 loads are
    contiguous (<512B-desc DMA penalty avoided); o-chain all bf16
    (fp32 matmul costs 4 cyc/row); out staged bf16, per-tile SP writes.

Performance map (TimelineSim cost model = the graded metric; HW-verified
rel_err 6.0e-3): 336us total (was 403).  Busy: DMA 237 (q/v/k gathers
175 = bandwidth-exact floor at 1.42ns/row, phase1 ~45, rest small),
DVE 184, PE 152, ACT 89, Pool 72.  B_RUN=32 (1 run/tile) was the big
unlock (-33us): fewer serialization rounds; B_RUN=16 phase-locked at
14.6us/run regardless of pool depths (gat/gatv/work depth all FLAT).
Remaining known gaps: ~2.3us before each esc-transpose (qk->PE->exp
latency) + ~4us before each tile's gathers + ~12us tail drain; total
DMA idle ~98us.  Tested flat/negative this session: SWDGE ring 32/64KB
(costs SBUF), chunked qk (neutral), out-writes on SP during phase 1
(-12us regression), in-place m into v_g (neutral, kept: frees 24KB),
per-chunk qks pool (enabled B_RUN=32 to fit).
Rejected by analysis: k-gather elimination via one-hot PE expansion
(PE+copy taxes ~+100us at mid-pstate > 58us DMA win; S_T needs a
PE transpose + PSUM->SBUF copy, no cheap j-on-partition one-hot);
e-layout score path (TensorReduce has NO 2x mode); fp8 gathers (<512B
desc penalty exactly cancels the bandwidth win; fp8 also kills DVE 2x);
DoubleRow segsum (needs both operands fp8; messages must stay bf16).
HW hazard ledger (inherited): single_packet gathers beyond ~256 idxs
corrupt; phase-1 pin bufs=4 NaN'd on HW while passing CoreSim.
"""
import math
from contextlib import ExitStack

import ml_dtypes
import numpy as np

import concourse.bass as bass
import concourse.mybir as mybir
import concourse.tile as tile
from concourse import bacc, bass_utils

F32 = mybir.dt.float32
BF16 = mybir.dt.bfloat16
I16 = mybir.dt.int16

N_NODES = 10000
DIM = 256
H = 8
HD = 32
NCORES = 8
NPC = N_NODES // NCORES          # nodes per core (1250)
W = 128                          # node-tile width
NT = (NPC + W - 1) // W          # node tiles per core (10)
N_CPAD = NT * W                  # padded local nodes (1280)
N_PAD = 10240                    # padded q/v table rows (80 tiles of 128)
B_RUN = 32                       # edge blocks (of 128 edges) per inner run
# v-table column permutation: position d*H+h holds head-h dim-d, so the
# attn broadcast in the message multiply lands on a middle dim and the
# last dim stays packed (2x DVE mode legal)
V_PERM = (np.arange(DIM) % H) * HD + np.arange(DIM) // H
# q/k-table column permutation: head h owns columns [h*16, h*16+16) and
# [128+h*16, 128+h*16+16), so the two 128-partition halves of q*k can be
# summed elementwise first (one DVE add) and the per-head reduction needs
# a single 128-contraction matmul per chunk instead of two
_c = np.arange(128)
QK_PERM = np.concatenate([(_c // 16) * HD + _c % 16,
                          (_c // 16) * HD + 16 + _c % 16])

MULT = mybir.AluOpType.mult
ADD = mybir.AluOpType.add
ISEQ = mybir.AluOpType.is_equal
AXX = mybir.AxisListType.X

last_results = None  # BassKernelResults of the most recent run (for test.py)


def _preprocess(src, dst):
    """Balance dst nodes across the 80 (core, tile) bins by degree (LPT)
    so the max per-bin edge count -- and thus the shared padding B -- is
    minimal, then bucket edges per bin and pad to B_RUN-aligned blocks.

    Returns the bin assignment maps needed to scatter per-core outputs
    back to original node order."""
    import heapq

    src = np.asarray(src).astype(np.int64)
    dst = np.asarray(dst).astype(np.int64)
    NB = NCORES * NT
    deg = np.bincount(dst, minlength=N_NODES)
    order = np.argsort(-deg, kind="stable")
    heap = [(0, b) for b in range(NB)]
    heapq.heapify(heap)
    cnt = np.zeros(NB, np.int64)
    ncnt = np.zeros(NB, np.int64)
    bin_of = np.zeros(N_NODES, np.int64)
    slot_of = np.zeros(N_NODES, np.int64)
    for v in order:
        held = []
        c, b = heapq.heappop(heap)
        while ncnt[b] >= W:
            held.append((c, b))
            c, b = heapq.heappop(heap)
        bin_of[v] = b
        slot_of[v] = ncnt[b]
        cnt[b] = c + deg[v]
        ncnt[b] += 1
        heapq.heappush(heap, (cnt[b], b))
        for e in held:
            heapq.heappush(heap, e)

    maxcnt = int(cnt.max())
    nruns = max(1, (maxcnt + B_RUN * 128 - 1) // (B_RUN * 128))
    B = B_RUN * nruns
    EPT = B * 128  # padded edges per node tile

    # row of each node inside its core's output block / k-table
    row_of = (bin_of % NT) * W + slot_of
    core_of_node = bin_of // NT

    e_bin = bin_of[dst]
    order_e = np.argsort(e_bin, kind="stable")
    s_src = src[order_e]
    s_dst = dst[order_e]
    s_bin = e_bin[order_e]

    src_pad = np.zeros((NCORES, NT, EPT), np.int64)
    kdst_pad = np.zeros((NCORES, NT, EPT), np.int64)     # local dst (k-table row)
    dstloc_pad = np.full((NCORES, NT, EPT), -1.0, np.float32)  # within-tile dst

    seg = np.searchsorted(s_bin, np.arange(NB + 1))
    for c in range(NCORES):
        for t in range(NT):
            i = c * NT + t
            lo, hi = seg[i], seg[i + 1]
            n = hi - lo
            assert n <= EPT
            src_pad[c, t, :n] = s_src[lo:hi]
            kdst_pad[c, t, :n] = row_of[s_dst[lo:hi]]
            dstloc_pad[c, t, :n] = slot_of[s_dst[lo:hi]].astype(np.float32)

    def tile_idx(a):
        # sequence -> dma_gather layout [128, S/16]: row p holds seq[s*16 + p%16]
        seq = a.reshape(-1, 16).T.astype(np.int16)       # [16, S/16]
        return np.ascontiguousarray(np.tile(seq, (8, 1)))  # [128, S/16]

    idx_src = np.stack([tile_idx(src_pad[c]) for c in range(NCORES)])
    idx_dst = np.stack([tile_idx(kdst_pad[c]) for c in range(NCORES)])
    # [128, NT*B] with [e, t*B+b] = dstloc[t, b*128+e]
    dstloc = np.stack([
        np.ascontiguousarray(
            dstloc_pad[c].reshape(NT, B, 128).transpose(2, 0, 1).reshape(128, NT * B))
        for c in range(NCORES)])
    return B, idx_src, idx_dst, dstloc, core_of_node, row_of


_prog_cache = {}


def _build(B):
    import os
    skip = set(os.environ.get("KERNEL_SKIP", "").split(","))
    nruns = B // B_RUN
    SEQ = NT * B * 128
    nc = bacc.Bacc("TRN2", target_bir_lowering=False, debug=False,
                   dynamic_dma_scratch_size=16384)

    xT_d = nc.dram_tensor("xT", [128, 2, N_PAD // 128, 128], BF16,
                          kind="ExternalInput").ap()
    xlocT_d = nc.dram_tensor("xlocT", [128, 2, N_CPAD // 128, 128], BF16,
                             kind="ExternalInput").ap()
    wqvT_d = nc.dram_tensor("wqvT", [DIM, 2 * DIM], BF16, kind="ExternalInput").ap()
    wkT_d = nc.dram_tensor("wkT", [DIM, DIM], BF16, kind="ExternalInput").ap()
    woT_d = nc.dram_tensor("woT", [DIM, DIM], BF16, kind="ExternalInput").ap()
    idxs_d = nc.dram_tensor("idx_src", [128, SEQ // 16], I16, kind="ExternalInput").ap()
    idxd_d = nc.dram_tensor("idx_dst", [128, SEQ // 16], I16, kind="ExternalInput").ap()
    dstloc_d = nc.dram_tensor("dstloc", [128, NT * B], BF16, kind="ExternalInput").ap()
    ident_d = nc.dram_tensor("ident", [128, 128], BF16, kind="ExternalInput").ap()
    bd16_d = nc.dram_tensor("bd16", [128, 16], BF16, kind="ExternalInput").ap()
    iota_d = nc.dram_tensor("iota", [128, 128 * B_RUN], BF16, kind="ExternalInput").ap()
    out_d = nc.dram_tensor("out", [N_CPAD, DIM], BF16, kind="ExternalOutput").ap()

    with ExitStack() as ctx:
        tc = ctx.enter_context(tile.TileContext(nc))
        consts = ctx.enter_context(tc.tile_pool(name="consts", bufs=1))

        def load_w(name, d_ap):
            sb = consts.tile([128, 2, d_ap.shape[1]], d_ap.dtype, name=name)
            nc.sync.dma_start(sb[:], d_ap.rearrange("(a p) i -> p a i", p=128))
            return sb

        wqv_sb = load_w("wqv_sb", wqvT_d)
        wk_sb = load_w("wk_sb", wkT_d)
        wo_sb = load_w("wo_sb", woT_d)
        ident = consts.tile([128, 128], BF16)
        nc.sync.dma_start(ident[:], ident_d)
        bd16 = consts.tile([128, 16], BF16)
        nc.sync.dma_start(bd16[:], bd16_d)
        iotab_sb = consts.tile([128, 128 * B_RUN], BF16)
        nc.sync.dma_start(iotab_sb[:], iota_d)
        idxs_sb = consts.tile([128, SEQ // 16], I16)
        nc.sync.dma_start(idxs_sb[:], idxs_d)
        idxd_sb = consts.tile([128, SEQ // 16], I16)
        nc.sync.dma_start(idxd_sb[:], idxd_d)
        dstloc_sb = consts.tile([128, NT * B], BF16)
        nc.sync.dma_start(dstloc_sb[:], dstloc_d)
        out_stage = consts.tile([128, NT, DIM], BF16)

        dram = ctx.enter_context(tc.tile_pool(name="dram", bufs=1, space="DRAM"))
        qv_table = dram.tile([N_PAD, 2 * DIM], BF16)
        k_table = dram.tile([N_CPAD, DIM], BF16)

        # ---- phase 1: projection tables ----
        with tc.tile_pool(name="pin", bufs=3) as pin, \
             tc.tile_pool(name="pps", bufs=4, space="PSUM") as pps, \
             tc.tile_pool(name="pout", bufs=4) as pout:

            def project(src_ap, n_tiles, jobs, table, width, grp):
                assert n_tiles % grp == 0
                x4 = src_ap.rearrange("p a (g t) w -> p a g t w", t=grp)
                tb = table[:].rearrange("(g t p) w -> p g t w", p=128, t=grp)
                for g in range(n_tiles // grp):
                    xt = pin.tile([128, 2, grp, 128], BF16, tag="xt")
                    nc.sync.dma_start(xt[:], x4[:, :, g, :, :])
                    ob = pout.tile([128, grp, width], BF16, tag="ob")
                    for t in range(grp):
                        ps = pps.tile([128, width], F32, tag="ps")
                        nc.tensor.matmul(ps[:], xt[:, 0, t, :], jobs[:, 0, :],
                                         start=True, stop=False)
                        nc.tensor.matmul(ps[:], xt[:, 1, t, :], jobs[:, 1, :],
                                         start=False, stop=True)
                        if t % 2 == 0:
                            nc.scalar.copy(ob[:, t, :], ps[:])
                        else:
                            nc.vector.tensor_copy(ob[:, t, :], ps[:])
                    nc.scalar.dma_start(tb[:, g, :, :], ob[:])

            if "phase1" not in skip:
                project(xlocT_d, NT, wk_sb[:], k_table, DIM, 5)
                project(xT_d, N_PAD // 128, wqv_sb[:], qv_table, 2 * DIM, 16)

        # ---- phase 2: per node-tile edge processing ----
        nidx_reg = nc.alloc_register(mybir.EngineType.Pool, "nidx_reg")
        nc.gpsimd.reg_mov(nidx_reg, B_RUN * 128)
        with tc.tile_pool(name="gat", bufs=3) as gat, \
             tc.tile_pool(name="gatv", bufs=2) as gatv, \
             tc.tile_pool(name="work", bufs=2) as work, \
             tc.tile_pool(name="qkp", bufs=2) as qkp, \
             tc.tile_pool(name="qksp", bufs=4) as qksp, \
             tc.tile_pool(name="spool", bufs=2) as spool, \
             tc.tile_pool(name="small", bufs=3) as small, \
             tc.tile_pool(name="hps", bufs=2, space="PSUM") as hps, \
             tc.tile_pool(name="sps", bufs=2, space="PSUM") as sps, \
             tc.tile_pool(name="tps", bufs=2, space="PSUM") as tps, \
             tc.tile_pool(name="stage", bufs=1) as stage:

            ne = B_RUN * 128
            ncols = B_RUN * 8
            nidx = B_RUN * 128
            for t in range(NT):
                h_ps = hps.tile([128, DIM], F32, tag="h")
                # k/q gathers for this tile (k first: k_table is built
                # before the qv table, so k-gathers can fill phase-1 idle)
                col0t = t * B * 8
                kT_g0 = gat.tile([128, 2, ne], BF16, tag="kTg")
                qT_g0 = gat.tile([128, 2, ne], BF16, tag="qTg")
                if "gather" not in skip:
                    nc.gpsimd.dma_gather(kT_g0[:], k_table[:],
                                         idxd_sb[:, col0t:col0t + ncols],
                                         nidx, nidx_reg, DIM,
                                         transpose=True, single_packet=False)
                    nc.gpsimd.dma_gather(qT_g0[:], qv_table[:, 0:DIM],
                                         idxs_sb[:, col0t:col0t + ncols],
                                         nidx, nidx_reg, DIM,
                                         elem_step=2 * DIM, transpose=True,
                                         single_packet=False)
                ms, Ss = [], []
                for r in range(nruns):
                    col0 = (t * B + r * B_RUN) * 8
                    kT_g, qT_g = kT_g0, qT_g0

                    # S depends only on constants: build early so DVE has
                    # work while the gathers land
                    S = spool.tile([128, 128, B_RUN], BF16, tag="S")
                    nc.vector.tensor_tensor(
                        S[:],
                        iotab_sb[:].rearrange("p (n b) -> p n b", b=B_RUN),
                        dstloc_sb[:, t * B + r * B_RUN:t * B + (r + 1) * B_RUN]
                            .unsqueeze(1).broadcast_to((128, 128, B_RUN)),
                        op=ISEQ)
                    Ss.append(S)

                    v_g = gatv.tile([128, B_RUN, DIM], BF16, tag="vg")
                    if "gather" not in skip:
                        nc.gpsimd.dma_gather(v_g[:], qv_table[:, DIM:2 * DIM],
                                             idxs_sb[:, col0:col0 + ncols],
                                             nidx, nidx_reg, DIM,
                                             elem_step=2 * DIM,
                                             single_packet=False)

                    if "compute" in skip:
                        continue
                    # q/k columns are half-head permuted: cols d and 128+d
                    # belong to the same head, so fold the halves first and
                    # use a single 128-contraction matmul per chunk; chunked
                    # so PE/ACT start while DVE is still on later chunks
                    escT = work.tile([16, ne], BF16, tag="escT")
                    for e0 in range(0, ne, 512):
                        sz = min(512, ne - e0)
                        qks = qksp.tile([128, 512], BF16, tag="qks")
                        for f0 in range(0, sz, 256):
                            qkc = qkp.tile([128, 2, 256], BF16, tag="qkc")
                            nc.vector.tensor_tensor(qkc[:],
                                                    qT_g[:, :, e0 + f0:e0 + f0 + 256],
                                                    kT_g[:, :, e0 + f0:e0 + f0 + 256],
                                                    op=MULT)
                            nc.vector.tensor_tensor(qks[:, f0:f0 + 256],
                                                    qkc[:, 0, :], qkc[:, 1, :],
                                                    op=ADD)
                        ps16 = sps.tile([16, 512], F32, tag="ps16")
                        nc.tensor.matmul(ps16[:, :sz], bd16[:], qks[:, :sz],
                                         start=True, stop=True)
                        nc.scalar.activation(escT[:, e0:e0 + sz], ps16[:, :sz],
                                             func=mybir.ActivationFunctionType.Exp,
                                             scale=1.0 / math.sqrt(HD))
                    esc_e = small.tile([128, B_RUN, 16], BF16, tag="esce")
                    nc.sync.dma_start(esc_e[:], escT[:], transpose=True)
                    z = small.tile([128, B_RUN], F32, tag="z")
                    nc.vector.tensor_reduce(z[:], esc_e[:, :, 0:H], axis=AXX, op=ADD)
                    zr = small.tile([128, B_RUN], F32, tag="zr")
                    nc.vector.reciprocal(zr[:], z[:])
                    attn = small.tile([128, B_RUN, H], BF16, tag="at")
                    nc.vector.tensor_tensor(
                        attn[:], esc_e[:, :, 0:H],
                        zr[:].unsqueeze(2).broadcast_to((128, B_RUN, H)), op=MULT)
                    # v table columns are (d, h)-permuted, so attn broadcasts
                    # over the middle dim and all last dims stay packed -> 2x;
                    # in-place into v_g (same-index elementwise) saves a pool
                    nc.vector.tensor_tensor(
                        v_g[:].rearrange("p b (d h) -> p b d h", h=H),
                        v_g[:].rearrange("p b (d h) -> p b d h", h=H),
                        attn[:].unsqueeze(2).broadcast_to((128, B_RUN, HD, H)),
                        op=MULT)
                    ms.append(v_g)
                # segment-sum after all runs' score pipelines are emitted:
                # keeps the in-order PE from blocking run r+1's score matmuls
                # behind run r's m-multiply
                if "compute" not in skip:
                    for r in range(nruns):
                        for b in range(B_RUN):
                            nc.tensor.matmul(h_ps[:], Ss[r][:, :, b],
                                             ms[r][:, b, :],
                                             start=(r == 0 and b == 0),
                                             stop=(r == nruns - 1 and b == B_RUN - 1))

                if "compute" in skip:
                    nc.vector.memset(out_stage[:, t, :], 0.0)
                    continue
                h_sb = stage.tile([128, DIM], BF16, tag="h_sb")
                nc.scalar.copy(h_sb[:], h_ps[:])
                hT_ps = tps.tile([128, 2, 128], BF16, tag="hT")
                for a in range(2):
                    nc.tensor.transpose(hT_ps[:, a, :],
                                        h_sb[:, a * 128:(a + 1) * 128], ident[:])
                hT_sb = stage.tile([128, 2, 128], BF16, tag="hT_sb")
                nc.scalar.copy(hT_sb[:], hT_ps[:])
                o_ps = tps.tile([128, DIM], F32, tag="o")
                for a in range(2):
                    nc.tensor.matmul(o_ps[:], hT_sb[:, a, :], wo_sb[:, a, :],
                                     start=(a == 0), stop=(a == 1))
                nc.scalar.copy(out_stage[:, t, :], o_ps[:])
                nc.sync.dma_start(
                    out_d.rearrange("(t p) w -> p t w", p=128)[:, t, :],
                    out_stage[:, t, :])


    nc.compile()
    return nc


def _bd_mat(base):
    bd = np.zeros((128, 16), np.float32)
    for d in range(128):
        bd[d, base + d // HD] = 1.0
    return bd.astype(ml_dtypes.bfloat16)


def _arr_x(xp):
    """[N, DIM] -> the phase-1 load layout [128 p, 2 a, N/128 t, 128 w]
    stored contiguously (one flat DMA per group load)."""
    n = xp.shape[0]
    xT = xp.T.astype(ml_dtypes.bfloat16)           # [DIM, N]
    return np.ascontiguousarray(
        xT.reshape(2, 128, n // 128, 128).transpose(1, 0, 2, 3))


def _make_in_maps(x, Wq, Wk, Wv, Wo, idx_src, idx_dst, dstloc,
                  core_of_node, row_of):
    x = np.asarray(x, np.float32)
    xp = np.zeros((N_PAD, DIM), np.float32)
    xp[:N_NODES] = x
    # v columns permuted to (d, h) order; Wo rows permuted to match;
    # q/k columns permuted for the folded score reduction
    wqvT = np.ascontiguousarray(np.concatenate(
        [np.asarray(Wq, np.float32).T[:, QK_PERM],
         np.asarray(Wv, np.float32).T[:, V_PERM]],
        axis=1).astype(ml_dtypes.bfloat16))
    wkT = np.ascontiguousarray(np.asarray(Wk, np.float32).T[:, QK_PERM]
                               .astype(ml_dtypes.bfloat16))
    woT = np.ascontiguousarray(np.asarray(Wo, np.float32).T[V_PERM, :]
                               .astype(ml_dtypes.bfloat16))
    bd16 = np.zeros((128, 16), np.float32)
    bd16[np.arange(128), np.arange(128) // 16] = 1.0
    in_maps = []
    for c in range(NCORES):
        xl = np.zeros((N_CPAD, DIM), np.float32)
        mine = core_of_node == c
        xl[row_of[mine]] = x[mine]
        in_maps.append({
            "xT": _arr_x(xp),
            "xlocT": _arr_x(xl),
            "wqvT": wqvT, "wkT": wkT, "woT": woT,
            "idx_src": idx_src[c], "idx_dst": idx_dst[c],
            "dstloc": dstloc[c].astype(ml_dtypes.bfloat16),
            "ident": np.eye(128, dtype=ml_dtypes.bfloat16),
            "bd16": bd16.astype(ml_dtypes.bfloat16),
            "iota": np.tile(np.repeat(np.arange(128), B_RUN)
                            .astype(ml_dtypes.bfloat16), (128, 1)),
        })
    return in_maps


def kernel(x, src, dst, Wq, bq, Wk, bk, Wv, bv, Wo, bo, **_unused):
    global last_results
    assert abs(np.asarray(bq)).max() == 0 and abs(np.asarray(bk)).max() == 0 \
        and abs(np.asarray(bv)).max() == 0, "nonzero qkv biases unsupported"

    B, idx_src, idx_dst, dstloc, core_of_node, row_of = _preprocess(src, dst)
    if B not in _prog_cache:
        _prog_cache[B] = _build(B)
    nc = _prog_cache[B]
    in_maps = _make_in_maps(x, Wq, Wk, Wv, Wo, idx_src, idx_dst, dstloc,
                            core_of_node, row_of)

    import os
    trace = bool(int(os.environ.get("KERNEL_TRACE", "0")))
    res = bass_utils.run_bass_kernel_spmd(
        nc, in_maps, core_ids=list(range(NCORES)), trace=trace)
    last_results = res

    allout = np.stack([np.asarray(res.results[c]["out"], np.float32)
                       for c in range(NCORES)])
    out = allout[core_of_node, row_of]
    out += np.asarray(bo, np.float32)[None, :]
    return out

